# revision 6
# baseline (speedup 1.0000x reference)
"""Causal self-attention (B=2, S=2048, E=1024, H=16) on 8 TRN2 NeuronCores.

Sharding: core i handles batch b = i//4 and head-group g = i%4 (4 heads,
256 features). Each core computes Q^T/K^T (feature-major), V (token-major,
with an appended ones-column so the softmax denominator falls out of the
attention@V matmul), causal softmax without max-subtraction (scores are
bounded for this distribution), and its partial o-projection. Host sums the
4 partials per batch and adds o_b.
"""

import functools
import math
import os
import sys

import numpy as np

for _p in ("/opt/trn_rl_repo",):
    if _p not in sys.path:
        sys.path.insert(0, _p)

import concourse.bass as bass
import concourse.mybir as mybir
import concourse.tile as tile
from concourse import bacc
from concourse.bass_utils import run_bass_kernel_spmd

P = 128          # partitions
S = 2048         # sequence
E = 1024         # model dim
F = 256          # features per core (4 heads x 64)
D = 64           # head dim
NH = 4           # heads per core
EC = E // P      # e-chunks (8)
SM = 512         # s-macro width
NJ = S // SM     # s-macros (4)
NSB = SM // P    # 128-blocks per s-macro (4)
NTB = S // P     # total 128-token blocks (16)

f32 = mybir.dt.float32
f32r = mybir.dt.float32r
ADD = mybir.AluOpType.add
EXP = mybir.ActivationFunctionType.Exp


def _r(ap):
    return ap


@functools.lru_cache(maxsize=1)
def _build():
    nc = bacc.Bacc("TRN2", target_bir_lowering=False, debug=False)

    xt = nc.dram_tensor("xt", [EC, P, S], f32r, kind="ExternalInput").ap()
    wq = nc.dram_tensor("wq", [P, EC, F], f32r, kind="ExternalInput").ap()
    wk = nc.dram_tensor("wk", [P, EC, F], f32r, kind="ExternalInput").ap()
    wv = nc.dram_tensor("wv", [P, EC, F], f32r, kind="ExternalInput").ap()
    wo = nc.dram_tensor("wo", [P, 2, E], f32r, kind="ExternalInput").ap()
    qb = nc.dram_tensor("qb", [P, 2], f32, kind="ExternalInput").ap()
    kb = nc.dram_tensor("kb", [P, 2], f32, kind="ExternalInput").ap()
    vb = nc.dram_tensor("vb", [P, F], f32, kind="ExternalInput").ap()
    vones = nc.dram_tensor("vones", [P, NTB, NH, 1], f32r, kind="ExternalInput").ap()
    out = nc.dram_tensor("out", [S, E], f32, kind="ExternalOutput").ap()

    with tile.TileContext(nc) as tc:
        with (
            tc.tile_pool(name="consts", bufs=1) as consts,
            tc.tile_pool(name="xtp", bufs=12) as xtp,
            tc.tile_pool(name="qtp", bufs=2) as qtp,
            tc.tile_pool(name="atp", bufs=2) as atp,
            tc.tile_pool(name="ptp", bufs=6) as ptp,
            tc.tile_pool(name="outp", bufs=3) as outp,
            tc.tile_pool(name="recipp", bufs=3) as recipp,
            tc.tile_pool(name="bcp", bufs=3) as bcp,
            tc.tile_pool(name="drp", bufs=3, space="DRAM") as drp,
            tc.tile_pool(name="ps", bufs=5, space="PSUM") as ps,
            tc.tile_pool(name="avps", bufs=2, space="PSUM") as avps,
        ):
            wq_s = consts.tile([P, EC, F], f32r, name="wq_s")
            wk_s = consts.tile([P, EC, F], f32r, name="wk_s")
            wv_s = consts.tile([P, EC, F], f32r, name="wv_s")
            wo_s = consts.tile([P, 2, E], f32r, name="wo_s")
            qb_s = consts.tile([P, 2], f32, name="qb_s")
            kb_s = consts.tile([P, 2], f32, name="kb_s")
            vb_s = consts.tile([P, F], f32, name="vb_s")
            nc.sync.dma_start(wq_s, wq)
            nc.sync.dma_start(wk_s, wk)
            nc.sync.dma_start(wv_s, wv)
            nc.sync.dma_start(wo_s, wo)
            nc.sync.dma_start(qb_s, qb)
            nc.sync.dma_start(kb_s, kb)
            nc.sync.dma_start(vb_s, vb)

            # K^T [f, s] and V [t, head, 65] persist across the whole kernel.
            kt_s = consts.tile([P, 2, S], f32r, name="kt_s")
            v_s = consts.tile([P, NTB, NH, D + 1], f32r, name="v_s")
            nc.sync.dma_start(v_s[:, :, :, D : D + 1], vones)

            qt_tiles = {}

            def proj(j):
                """QKV projections for s-macro j."""
                sj = slice(j * SM, (j + 1) * SM)
                xts = []
                for c in range(EC):
                    xc = xtp.tile([P, SM], f32r, name="xc")
                    nc.sync.dma_start(xc, xt[c, :, sj])
                    xts.append(xc)

                qt = qtp.tile([P, 2, SM], f32r, name="qt")
                qt_tiles[j] = qt
                for w_s, b_s, dest in (
                    (wq_s, qb_s, qt),
                    (wk_s, kb_s, kt_s[:, :, sj]),
                ):
                    for ft in range(2):
                        mm = ps.tile([P, SM], f32, name="mmps")
                        for c in range(EC):
                            nc.tensor.matmul(
                                mm,
                                lhsT=_r(w_s[:, c, ft * P : (ft + 1) * P]),
                                rhs=_r(xts[c]),
                                start=(c == 0),
                                stop=(c == EC - 1),
                            )
                        nc.vector.tensor_scalar(
                            out=dest[:, ft, :],
                            in0=mm,
                            scalar1=b_s[:, ft : ft + 1],
                            scalar2=None,
                            op0=ADD,
                        )

                for sb in range(NSB):
                    mm = ps.tile([P, SM], f32, name="mmps")
                    vv = mm[:, :F]
                    for c in range(EC):
                        nc.tensor.matmul(
                            vv,
                            lhsT=_r(xts[c][:, sb * P : (sb + 1) * P]),
                            rhs=_r(wv_s[:, c, :]),
                            start=(c == 0),
                            stop=(c == EC - 1),
                        )
                    tb = j * NSB + sb
                    for h in range(NH):
                        nc.vector.tensor_add(
                            v_s[:, tb, h, 0:D],
                            vv[:, h * D : (h + 1) * D],
                            vb_s[:, h * D : (h + 1) * D],
                        )

            def attn(j):
                """Attention for query s-macro j, all 4 heads."""
                sj = slice(j * SM, (j + 1) * SM)
                qt = qt_tiles[j]
                at = atp.tile([P, 2, SM], f32r, name="at")
                ktiles = NSB * (j + 1)
                LAG = 3
                for h in range(NH):
                    hp = (h % 2) * D
                    hc = h // 2
                    avp = avps.tile([D + 1, SM], f32, name="avp")
                    pts = {}
                    for step in range(ktiles + LAG):
                        if step < ktiles:
                            tb = step
                            mm = ps.tile([P, SM], f32, name="mmps")
                            nc.tensor.matmul(
                                mm,
                                lhsT=_r(kt_s[hp : hp + D, hc, tb * P : (tb + 1) * P]),
                                rhs=_r(qt[hp : hp + D, hc, :]),
                                start=True,
                                stop=True,
                            )
                            pt = ptp.tile([P, SM], f32r, name="pt")
                            nc.scalar.activation(pt, mm, EXP)
                            r = tb - NSB * j
                            if r >= 0:
                                # zero where t_local + 128*r > s_local
                                nc.gpsimd.affine_select(
                                    out=pt,
                                    in_=pt,
                                    compare_op=mybir.AluOpType.is_ge,
                                    fill=0.0,
                                    base=-(r * P),
                                    pattern=[[1, SM]],
                                    channel_multiplier=-1,
                                )
                            pts[tb] = pt
                        if step >= LAG:
                            tb = step - LAG
                            nc.tensor.matmul(
                                avp,
                                lhsT=_r(v_s[:, tb, h, :]),
                                rhs=_r(pts.pop(tb)),
                                start=(tb == 0),
                                stop=(tb == ktiles - 1),
                            )

                    recip = recipp.tile([1, SM], f32, name="recip")
                    nc.vector.reciprocal(recip, avp[D : D + 1, :])
                    bounce = drp.tile([1, SM], f32, name="bounce")
                    nc.sync.dma_start(bounce, recip)
                    bc = bcp.tile([D, SM], f32, name="bc")
                    nc.sync.dma_start(
                        bc,
                        bass.AP(
                            tensor=bounce.tensor,
                            offset=bounce.offset,
                            ap=[[0, D], [1, SM]],
                        ),
                    )
                    nc.vector.tensor_mul(
                        at[hp : hp + D, hc, :], avp[0:D, :], bc
                    )
                return at

            def oproj(j, at):
                """Partial o-projection for s-macro j."""
                for sb in range(NSB):
                    ob = outp.tile([P, E], f32, name="ob")
                    for eh in range(2):
                        mm = ps.tile([P, SM], f32, name="mmps")
                        for fc in range(2):
                            nc.tensor.matmul(
                                mm,
                                lhsT=_r(at[:, fc, sb * P : (sb + 1) * P]),
                                rhs=_r(wo_s[:, fc, eh * SM : (eh + 1) * SM]),
                                start=(fc == 0),
                                stop=(fc == 1),
                            )
                        nc.vector.tensor_copy(ob[:, eh * SM : (eh + 1) * SM], mm)
                    row = (j * NSB + sb) * P
                    nc.sync.dma_start(out[row : row + P, :], ob)

            proj(0)
            for j in range(NJ):
                at = attn(j)
                if j + 1 < NJ:
                    proj(j + 1)
                oproj(j, at)

    nc.finalize()  # Bacc: reg alloc, wait splitting, event semaphores
    return nc


def _in_maps(inputs):
    X = np.asarray(inputs["X"], np.float32)
    q_w = np.asarray(inputs["q_w"], np.float32)
    q_b = np.asarray(inputs["q_b"], np.float32)
    k_w = np.asarray(inputs["k_w"], np.float32)
    k_b = np.asarray(inputs["k_b"], np.float32)
    v_w = np.asarray(inputs["v_w"], np.float32)
    v_b = np.asarray(inputs["v_b"], np.float32)
    o_w = np.asarray(inputs["o_w"], np.float32)

    sc = 1.0 / math.sqrt(D)

    def swz(w):  # [E, F] -> [P, EC, F]
        return np.ascontiguousarray(w.reshape(EC, P, F).transpose(1, 0, 2))

    maps = []
    for core in range(8):
        b, g = divmod(core, 4)
        fr = slice(g * F, (g + 1) * F)
        xt = np.ascontiguousarray(X[b].T.reshape(EC, P, S))
        wq = swz(q_w[fr].T * sc)
        wk = swz(k_w[fr].T)
        wv = swz(v_w[fr].T)
        # wo: [F, E] -> [P, 2, E]
        wo = np.ascontiguousarray(o_w[:, fr].T.reshape(2, P, E).transpose(1, 0, 2))
        qb = np.ascontiguousarray((q_b[fr] * sc).reshape(2, P).T)
        kb = np.ascontiguousarray(k_b[fr].reshape(2, P).T)
        vb = np.ascontiguousarray(np.broadcast_to(v_b[fr], (P, F)))
        maps.append(
            {
                "xt": xt,
                "wq": wq,
                "wk": wk,
                "wv": wv,
                "wo": wo,
                "qb": qb,
                "kb": kb,
                "vb": vb,
                "vones": np.ones((P, NTB, NH, 1), np.float32),
            }
        )
    return maps


def run(inputs, **kwargs):
    nc = _build()
    res = run_bass_kernel_spmd(nc, _in_maps(inputs), core_ids=list(range(8)), **kwargs)
    o_b = np.asarray(inputs["o_b"], np.float32)
    outs = [res.results[i]["out"] for i in range(8)]
    full = np.empty((2, S, E), np.float32)
    for b in range(2):
        full[b] = outs[4 * b] + outs[4 * b + 1] + outs[4 * b + 2] + outs[4 * b + 3]
        full[b] += o_b
    return full, res


def kernel(**inputs):
    full, _ = run(inputs)
    return full
